# revision 1
# baseline (speedup 1.0000x reference)
"""Trainium2 Bass kernel for nn_CovarianceResidualError.

Computes, for errors [N, O] and graph_emb [N, D]:
    em   = errors - mean(errors, axis=0)
    a0   = (graph_emb - mean(graph_emb, axis=0))[:, :1]
    out  = -sum_o | sum_i em[i, o] * a0[i, 0] |

Identity used on device (exact in exact arithmetic):
    sum_i (e[i,o] - mean_e[o]) * (g[i] - mean_g)
      = sum_i e[i,o]*g[i]  -  mean_g * sum_i e[i,o]
(the mean_e term cancels because sum_i (g[i] - mean_g) == 0).

Sharding: data-parallel over N across 8 NeuronCores. Each core computes
partial P1[o] = sum_i e*g, P2[o] = sum_i e, s = sum_i g over its row shard
(PE matmul with a [g | 1] stationary weight pair per 128-row tile; e tiles
are down-converted to bf16 on the idle scalar/vector engines so the PE
streams at 1 cycle/row instead of f32's 4). The O-length signed partial
sums are reduced across cores BEFORE any abs: either by an on-device
8-core AllReduce (DEVICE_ALLREDUCE=True) or by the host-side gather
(default — an 8-core 2 KB mesh AllReduce costs a ~35 us latency floor on
the critical path, and the gather step has to read the outputs anyway).
abs and the final sum always happen after the global sum.
"""

import sys

if "/opt/trn_rl_repo" not in sys.path:
    sys.path.insert(0, "/opt/trn_rl_repo")

import numpy as np

import concourse.bacc as bacc
import concourse.mybir as mybir
import concourse.tile as tile
from concourse.bass_utils import run_bass_kernel_spmd

N, D, O = 131072, 128, 256
NCORES = 8
NLOC = N // NCORES          # 16384 rows per core
KP = 128                    # contraction (partition) dim per matmul
NT = NLOC // KP             # 128 sub-tiles per core
SUB = 8                     # sub-tiles per DMA -> 1 MiB per dma_start
NB = NT // SUB              # 16 big DMA tiles
EBUFS = 12                   # in-flight e tiles

# If True, the 8 cores AllReduce the [2*O+1] partials on device and every
# core emits the final scalar. If False, each core emits its partials and
# the host does the (tiny) 8-way combine — saves the ~37us mesh-collective
# latency floor that would sit exposed on the critical path.
DEVICE_ALLREDUCE = False

_nc_cache = {}


def _build(device_allreduce):
    f32 = mybir.dt.float32
    bf16 = mybir.dt.bfloat16
    nc = bacc.Bacc("TRN2", target_bir_lowering=False, debug=False,
                   num_devices=NCORES)
    e_ext = nc.dram_tensor("e", [NLOC, O], f32, kind="ExternalInput")
    g_ext = nc.dram_tensor("g", [NLOC, 1], f32, kind="ExternalInput")
    if device_allreduce:
        out_ext = nc.dram_tensor("out", [1], f32, kind="ExternalOutput")
    else:
        out_ext = nc.dram_tensor("out", [2 * O + 1], f32, kind="ExternalOutput")

    # Interleaved row tiling: sub-tile t uses rows {k*NT + t, k=0..127}, so
    # partition k streams contiguous DRAM rows and the per-tile weight
    # column is a natural-layout column of g.
    e_r = e_ext.rearrange("(k t) o -> k t o", k=KP)          # [128, 128, 256]
    g_r = g_ext.rearrange("(p f) one -> p (f one)", p=KP)    # [128, 128]

    with tile.TileContext(nc) as tc:
        with (
            tc.tile_pool(name="const", bufs=1) as cpool,
            tc.tile_pool(name="io", bufs=EBUFS) as iopool,
            tc.tile_pool(name="bf", bufs=4) as bpool,
            tc.tile_pool(name="small", bufs=1) as spool,
            tc.tile_pool(name="psum", bufs=1, space="PSUM") as ppool,
            tc.tile_pool(name="dram", bufs=1, space="DRAM") as dpool,
        ):
            # first e tile DMA goes first so the stream starts as early as
            # possible; nothing below gates it
            et0 = iopool.tile([KP, SUB, O], f32, tag="et")
            nc.sync.dma_start(out=et0[:], in_=e_r[:, 0:SUB, :])

            # g loads go via gpsimd SWDGE so the sync HWDGE ring (FIFO per
            # issuing engine) carries nothing but the e stream
            g_nat = cpool.tile([KP, NT], f32)                 # g_nat[k,t] = g[k*128+t]
            nc.gpsimd.dma_start(out=g_nat[:], in_=g_r)
            # W[:, 2t] = g column for sub-tile t, W[:, 2t+1] = 1.0 (bf16 so
            # the PE streams 1 cycle/row instead of 4 for f32)
            w = cpool.tile([KP, 2 * NT], bf16)
            nc.vector.memset(w[:], 1.0)
            nc.vector.tensor_copy(out=w[:, 0:2 * NT:2], in_=g_nat[:])

            # local sum of g: free-axis reduce, then partition fold via a
            # tiny SBUF->SBUF DMA into one row
            g_rowsum = spool.tile([KP, 1], f32)
            nc.vector.reduce_sum(out=g_rowsum[:], in_=g_nat[:],
                                 axis=mybir.AxisListType.X)
            g_row = spool.tile([1, KP], f32)
            nc.gpsimd.dma_start(out=g_row[:], in_=g_rowsum[:])
            s_sb = spool.tile([1, 1], f32)
            nc.vector.reduce_sum(out=s_sb[:], in_=g_row[:],
                                 axis=mybir.AxisListType.X)

            # main pass: psum[0,o] += sum_k g*e ; psum[1,o] += sum_k e
            # e tiles stream in as f32, get down-converted to bf16 on the
            # (otherwise idle) scalar engine, and the PE reduces in bf16
            # with f32 PSUM accumulation.
            psum_out = ppool.tile([2, O], f32)
            for b in range(NB):
                if b == 0:
                    et = et0
                else:
                    et = iopool.tile([KP, SUB, O], f32, tag="et")
                    nc.sync.dma_start(out=et[:], in_=e_r[:, b * SUB:(b + 1) * SUB, :])
                eb = bpool.tile([KP, SUB, O], bf16, tag="eb")
                # alternate the f32->bf16 down-convert between the two idle
                # elementwise engines so neither sits on the critical path
                if b % 2 == 0:
                    nc.scalar.copy(out=eb[:], in_=et[:])
                else:
                    nc.vector.tensor_copy(out=eb[:], in_=et[:])
                for j in range(SUB):
                    t = b * SUB + j
                    nc.tensor.matmul(
                        psum_out[:],
                        lhsT=w[:, 2 * t:2 * t + 2],
                        rhs=eb[:, j, :],
                        start=(t == 0),
                        stop=(t == NT - 1),
                    )

            # pack [P1 | P2 | s]
            # (DMA cannot read PSUM, so bounce through SBUF first)
            part_sb = spool.tile([2, O], f32)
            nc.vector.tensor_copy(out=part_sb[:], in_=psum_out[:])

            if not device_allreduce:
                # emit per-core partials; host does the 8-way combine
                # (partition-major flatten: row 0 = P1, row 1 = P2)
                nc.sync.dma_start(out=out_ext[0:2 * O], in_=part_sb[:])
                nc.sync.dma_start(out=out_ext[2 * O:2 * O + 1], in_=s_sb[:])
            else:
                cc_in = dpool.tile([1, 2 * O + 1], f32)
                cc_out = dpool.tile([1, 2 * O + 1], f32)
                nc.sync.dma_start(out=cc_in[0:1, 0:O], in_=part_sb[0:1, :])
                nc.sync.dma_start(out=cc_in[0:1, O:2 * O], in_=part_sb[1:2, :])
                nc.sync.dma_start(out=cc_in[0:1, 2 * O:2 * O + 1], in_=s_sb[:])
                nc.gpsimd.collective_compute(
                    "AllReduce",
                    mybir.AluOpType.add,
                    replica_groups=[list(range(NCORES))],
                    ins=[cc_in.opt()],
                    outs=[cc_out.opt()],
                )

                # finish: col = P1 - (s/N)*P2 ; out = -sum |col|
                fin = spool.tile([1, 2 * O + 1], f32)
                nc.sync.dma_start(out=fin[:], in_=cc_out[:])
                meang = spool.tile([1, 1], f32)
                nc.vector.tensor_scalar_mul(out=meang[:],
                                            in0=fin[0:1, 2 * O:2 * O + 1],
                                            scalar1=1.0 / N)
                tmp = spool.tile([1, O], f32)
                nc.vector.tensor_scalar_mul(out=tmp[:], in0=fin[0:1, O:2 * O],
                                            scalar1=meang[0:1, 0:1])
                col = spool.tile([1, O], f32)
                nc.vector.tensor_sub(out=col[:], in0=fin[0:1, 0:O], in1=tmp[:])
                res = spool.tile([1, 1], f32)
                nc.vector.reduce_sum(out=res[:], in_=col[:],
                                     axis=mybir.AxisListType.X,
                                     apply_absolute_value=True)
                nres = spool.tile([1, 1], f32)
                nc.vector.tensor_scalar_mul(out=nres[:], in0=res[:], scalar1=-1.0)
                nc.sync.dma_start(out=out_ext[0:1], in_=nres[0:1, 0:1])

    nc.compile()
    return nc


def _get_nc(device_allreduce):
    if device_allreduce not in _nc_cache:
        _nc_cache[device_allreduce] = _build(device_allreduce)
    return _nc_cache[device_allreduce]


def _make_in_maps(graph_emb, errors):
    errors = np.asarray(errors, dtype=np.float32)
    g = np.ascontiguousarray(np.asarray(graph_emb, dtype=np.float32)[:, 0:1])
    in_maps = []
    for c in range(NCORES):
        sl = slice(c * NLOC, (c + 1) * NLOC)
        in_maps.append({
            "e": np.ascontiguousarray(errors[sl]),
            "g": np.ascontiguousarray(g[sl]),
        })
    return in_maps


def _run(graph_emb, errors, device_allreduce=DEVICE_ALLREDUCE, **spmd_kwargs):
    nc = _get_nc(device_allreduce)
    in_maps = _make_in_maps(graph_emb, errors)
    return run_bass_kernel_spmd(nc, in_maps, list(range(NCORES)), **spmd_kwargs)


def _combine_partials(results):
    """8-way sum of per-core [P1 | P2 | s] partials, then
    col = P1 - (s/N)*P2 ; out = -sum |col|  (abs strictly after the
    global sum)."""
    acc = np.zeros(2 * O + 1, dtype=np.float64)
    for r in results:
        acc += r["out"].astype(np.float64)
    p1, p2, s = acc[0:O], acc[O:2 * O], acc[2 * O]
    col = p1 - (s / N) * p2
    return np.float32(-np.abs(col).sum())


def kernel(targets=None, out0=None, out1=None, graph_emb=None, errors=None,
           **_unused):
    res = _run(graph_emb, errors)
    if DEVICE_ALLREDUCE:
        val = np.float32(res.results[0]["out"][0])
    else:
        val = _combine_partials(res.results)
    return np.asarray(val, dtype=np.float32).reshape(())

